# revision 3
# baseline (speedup 1.0000x reference)
"""BERT-BiGRU-CRF loss kernel for 8 TRN2 NeuronCores.

Strategy (per sharding hint): data-parallel over batch. Each of the 8 cores
computes the dominant GEMM - the GRU input projections for both directions,
x[16*512, 768] @ Wcat[768, 384] - on the TensorEngine via a Bass/Tile kernel
wrapped in bass_jit + bass_shard_map, so the compiled SPMD executable is
built once at module scope and reused across kernel() calls (the generic
run_bass_kernel_spmd axon path re-traces and re-lowers the NEFF wrapper on
every invocation, which dominated the baseline's wall time).

The sequential parts (GRU over T with 64-wide hidden, CRF forward with 9
labels) run on host via a jax-CPU jitted scan, exactly mirroring the
reference math; the scalar mean loss is the final reduction.
"""

import numpy as np

B, T, HID = 128, 512, 768
H = 64
G3 = 3 * H            # 192
L = 9
NCORES = 8
BS = B // NCORES      # 16 sequences per core
M = BS * T            # 8192 rows per core
N = 2 * G3            # 384: [fwd gates | bwd gates]
K = HID

_C = {}


def _build():
    """Build device + host executables once; cache in _C."""
    import jax
    import jax.numpy as jnp
    from jax.sharding import Mesh, PartitionSpec as P
    import concourse.mybir as mybir
    from concourse import tile
    from concourse.bass2jax import bass_jit, bass_shard_map

    f32 = mybir.dt.float32
    KT = K // 128          # 6 k-tiles
    MG = M // 512          # 16 groups of 512 rows

    @bass_jit
    def proj_kernel(nc, xT, W):
        # xT: [K, M] per-core, W: [K, N] replicated; out[M, N] = xT.T @ W
        out = nc.dram_tensor("out", [M, N], f32, kind="ExternalOutput")
        with tile.TileContext(nc) as tc:
            with (
                tc.tile_pool(name="wp", bufs=1) as wp,
                tc.tile_pool(name="xp", bufs=2) as xp,
                tc.tile_pool(name="op", bufs=4) as op,
                tc.tile_pool(name="pp", bufs=4, space="PSUM") as pp,
            ):
                w_tiles = []
                for k in range(KT):
                    wt = wp.tile([128, N], f32, tag=f"w{k}")
                    nc.sync.dma_start(wt[:], W[k * 128:(k + 1) * 128, :])
                    w_tiles.append(wt)
                for mg in range(MG):
                    x_tiles = []
                    for k in range(KT):
                        xt = xp.tile([128, 512], f32, tag=f"x{k}")
                        nc.sync.dma_start(
                            xt[:], xT[k * 128:(k + 1) * 128, mg * 512:(mg + 1) * 512]
                        )
                        x_tiles.append(xt)
                    for sub in range(4):
                        ps = pp.tile([128, N], f32, tag="ps")
                        for k in range(KT):
                            nc.tensor.matmul(
                                ps[:],
                                x_tiles[k][:, sub * 128:(sub + 1) * 128],
                                w_tiles[k][:],
                                start=(k == 0),
                                stop=(k == KT - 1),
                            )
                        ot = op.tile([128, N], f32, tag="o")
                        nc.vector.tensor_copy(ot[:], ps[:])
                        m0 = mg * 512 + sub * 128
                        nc.sync.dma_start(out[m0:m0 + 128, :], ot[:])
        return out

    devices = jax.devices()[:NCORES]
    mesh = Mesh(np.asarray(devices), ("c",))
    sharded = bass_shard_map(
        proj_kernel, mesh=mesh, in_specs=(P("c"), P()), out_specs=P("c")
    )

    # ---- host-side GRU + CRF, jitted on CPU ----
    cpu = jax.devices("cpu")[0]

    def finish(proj, mask, label, b_ih_f, b_hh_f, W_hh_f, b_ih_b, b_hh_b,
               W_hh_b, W_lin, b_lin, start_trans, end_trans, trans):
        # proj: [B,T,384] = x @ [W_ih_f.T | W_ih_b.T]
        m = mask
        mf = m.astype(jnp.float32)
        mt = mf.T[:, :, None]                                   # [T,B,1]
        xp_f = proj[:, :, :G3].transpose(1, 0, 2) + b_ih_f      # [T,B,3H]
        xp_b = proj[:, :, G3:].transpose(1, 0, 2) + b_ih_b

        def gru(xp, mtd, W_hh, b_hh):
            def step(h, inp):
                xg, mtt = inp
                hg = h @ W_hh.T + b_hh
                r = jax.nn.sigmoid(xg[:, :H] + hg[:, :H])
                z = jax.nn.sigmoid(xg[:, H:2 * H] + hg[:, H:2 * H])
                n = jnp.tanh(xg[:, 2 * H:] + r * hg[:, 2 * H:])
                h_new = (1.0 - z) * n + z * h
                h = jnp.where(mtt > 0, h_new, h)
                return h, h * mtt
            h0 = jnp.zeros((xp.shape[1], H), xp.dtype)
            _, out = jax.lax.scan(step, h0, (xp, mtd))
            return out

        out_f = gru(xp_f, mt, W_hh_f, b_hh_f)
        out_b = gru(xp_b[::-1], mt[::-1], W_hh_b, b_hh_b)[::-1]
        feat = jnp.concatenate([out_f, out_b], -1).transpose(1, 0, 2)
        em = feat @ W_lin.T + b_lin                             # [B,T,L]

        em_sc = jnp.take_along_axis(em, label[..., None], -1)[..., 0]
        tr_sc = trans[label[:, :-1], label[:, 1:]]
        score = start_trans[label[:, 0]] + em_sc[:, 0] \
            + jnp.sum(mf[:, 1:] * (tr_sc + em_sc[:, 1:]), axis=1)
        last = jnp.sum(m.astype(jnp.int32), axis=1) - 1
        last_tag = jnp.take_along_axis(label, last[:, None], 1)[:, 0]
        score = score + end_trans[last_tag]

        def pstep(alpha, inp):
            em_t, m_t = inp
            nxt = jax.nn.logsumexp(
                alpha[:, :, None] + trans[None] + em_t[:, None, :], axis=1)
            return jnp.where(m_t[:, None], nxt, alpha), None
        alpha0 = start_trans + em[:, 0]
        alpha, _ = jax.lax.scan(
            pstep, alpha0, (em[:, 1:].transpose(1, 0, 2), m[:, 1:].T))
        logZ = jax.nn.logsumexp(alpha + end_trans, axis=-1)
        return -jnp.mean(score - logZ)

    with jax.default_device(cpu):
        finish_jit = jax.jit(finish)

    _C["sharded"] = sharded
    _C["finish"] = finish_jit
    _C["cpu"] = cpu
    _C["jax"] = jax
    return _C


def kernel(length, word2vec, mask, label, W_ih_f, W_hh_f, b_ih_f, b_hh_f,
           W_ih_b, W_hh_b, b_ih_b, b_hh_b, W_lin, b_lin,
           start_trans, end_trans, trans):
    word2vec = np.asarray(word2vec, np.float32)
    mask = np.asarray(mask)
    label = np.asarray(label)
    Wcat = np.ascontiguousarray(
        np.concatenate([np.asarray(W_ih_f).T, np.asarray(W_ih_b).T], axis=1),
        dtype=np.float32)

    import time as _time
    tlog = _C.setdefault("t", {})
    proj = None
    try:
        t0 = _time.perf_counter()
        if not _C or "sharded" not in _C:
            _build()
        t1 = _time.perf_counter()
        jax = _C["jax"]
        # per-core [K, M] stacked on axis 0 -> [8*K, M]
        xT_all = np.ascontiguousarray(
            word2vec.reshape(NCORES, M, K).transpose(0, 2, 1)
        ).reshape(NCORES * K, M)
        t2 = _time.perf_counter()
        out = _C["sharded"](xT_all, Wcat)       # [8*M, N]
        out.block_until_ready()
        t3 = _time.perf_counter()
        proj = np.asarray(out).reshape(B, T, N)
        t4 = _time.perf_counter()
        tlog.update(build=t1 - t0, transpose=t2 - t1, device=t3 - t2,
                    fetch=t4 - t3, dev_ok=True)
    except Exception as e:
        tlog.update(dev_ok=False, dev_err=repr(e)[:500])
        proj = (word2vec.reshape(B * T, K) @ Wcat).reshape(B, T, N)

    try:
        t5 = _time.perf_counter()
        jax = _C["jax"]
        cpu = _C["cpu"]
        dp = lambda a: jax.device_put(np.asarray(a), cpu)
        loss = _C["finish"](
            dp(proj), dp(mask), dp(label),
            dp(b_ih_f), dp(b_hh_f), dp(W_hh_f),
            dp(b_ih_b), dp(b_hh_b), dp(W_hh_b),
            dp(W_lin), dp(b_lin),
            dp(start_trans), dp(end_trans), dp(trans))
        loss = np.float32(loss)
        tlog.update(finish=_time.perf_counter() - t5, fin_ok=True)
        return loss
    except Exception as e:
        tlog.update(fin_ok=False, fin_err=repr(e)[:500])
        return _finish_np(
            proj, mask, label,
            np.asarray(b_ih_f), np.asarray(b_hh_f), np.asarray(W_hh_f),
            np.asarray(b_ih_b), np.asarray(b_hh_b), np.asarray(W_hh_b),
            np.asarray(W_lin), np.asarray(b_lin),
            np.asarray(start_trans), np.asarray(end_trans), np.asarray(trans))


# ---------- pure-numpy fallback (mirrors reference exactly) ----------

def _sigmoid(x):
    return 1.0 / (1.0 + np.exp(-x))


def _gru_dir_np(xp, m, W_hh, b_hh):
    Bn = xp.shape[1]
    h = np.zeros((Bn, H), np.float32)
    out = np.empty((T, Bn, H), np.float32)
    WhhT = W_hh.T.astype(np.float32)
    for t in range(T):
        hg = h @ WhhT + b_hh
        xg = xp[t]
        r = _sigmoid(xg[:, :H] + hg[:, :H])
        z = _sigmoid(xg[:, H:2 * H] + hg[:, H:2 * H])
        n = np.tanh(xg[:, 2 * H:] + r * hg[:, 2 * H:])
        h_new = (1.0 - z) * n + z * h
        mt = m[t]
        h = np.where(mt > 0, h_new, h)
        out[t] = h * mt
    return out


def _logsumexp_np(x, axis):
    mx = np.max(x, axis=axis, keepdims=True)
    return (mx + np.log(np.sum(np.exp(x - mx), axis=axis, keepdims=True))).squeeze(axis)


def _finish_np(proj, mask, label, b_ih_f, b_hh_f, W_hh_f,
               b_ih_b, b_hh_b, W_hh_b, W_lin, b_lin,
               start_trans, end_trans, trans):
    mf = mask.astype(np.float32)
    mt = mf.T[:, :, None]
    xp_f = proj[:, :, :G3].transpose(1, 0, 2) + b_ih_f
    xp_b = proj[:, :, G3:].transpose(1, 0, 2) + b_ih_b
    out_f = _gru_dir_np(xp_f, mt, W_hh_f, b_hh_f)
    out_b = _gru_dir_np(xp_b[::-1], mt[::-1], W_hh_b, b_hh_b)[::-1]
    feat = np.concatenate([out_f, out_b], -1).transpose(1, 0, 2)
    em = feat @ W_lin.T + b_lin

    em_sc = np.take_along_axis(em, label[..., None], -1)[..., 0]
    tr_sc = trans[label[:, :-1], label[:, 1:]]
    score = start_trans[label[:, 0]] + em_sc[:, 0] \
        + np.sum(mf[:, 1:] * (tr_sc + em_sc[:, 1:]), axis=1)
    last = mask.astype(np.int64).sum(1) - 1
    last_tag = label[np.arange(label.shape[0]), last]
    score = score + end_trans[last_tag]

    alpha = start_trans + em[:, 0]
    for t in range(1, T):
        nxt = _logsumexp_np(
            alpha[:, :, None] + trans[None] + em[:, t][:, None, :], axis=1)
        alpha = np.where(mask[:, t][:, None], nxt, alpha)
    logZ = _logsumexp_np(alpha + end_trans, axis=-1)
    return np.float32(-(score - logZ).mean())
